# revision 21
# baseline (speedup 1.0000x reference)
"""Bass/Trainium2 kernel for ExtendedTripletLoss (data-parallel over batch).

FP8 redesign. Math per sample and shift off in [-4,4], pair g in {ap, an}:
  num(off) = t1 + t2 - 2*t3
    t3 = sum_{r,w} U[r,w] V[r,w-off],  U = m1*f1, V = m2*f2   (fp8)
    t1 = sum_{h,w} m1[h,w] m2[h,w-off] P1[h,w],   P1 = sum_c f1^2
    t2 = sum_{h,w} m1[h,w] m2[h,w-off] P2[h,w-off], P2 = sum_c f2^2
  den(off) = C * |m1 & m2r| + 1e-3

On-chip (per sample): one blob DMA carries fp8 a/p/n (chunk-major
[128, 4, 512]) plus 0xFF/0x00 mask bytes. Masking is a DVE bitwise-AND on
uint16 views (2x mode -> 4 fp8/cycle). Unmasked squares run split across
ScalarE/DVE/GpSimd. All matmuls are DoubleRow fp8 (K=256/step): an
indicator matmul reduces squares over c into P-maps [32, 3, 512], and the
U.V Gram accumulates band blocks into PSUM [128, 2, 136]. Both PSUM tiles
DMA straight to DRAM. Host extracts the 9 band diagonals (t3), applies
masks to the P-maps (t1/t2 - same O(B*H*W*9) class as the den counts),
and finishes min/relu/mean in f64.
"""

import os
import sys
from contextlib import ExitStack

import numpy as np

for _p in ("/opt/trn_rl_repo", "/root/.axon_site/_ro/trn_rl_repo"):
    if os.path.isdir(_p) and _p not in sys.path:
        sys.path.insert(0, _p)
        break

import ml_dtypes

import concourse.bass as bass
import concourse.mybir as mybir
import concourse.tile as tile

# This environment's walrus_driver allows only ONE sync-wait per instruction,
# while Tile freely aggregates several. Post-pass: move excess waits onto
# freshly inserted same-engine NOPs directly before the instruction.
_MAXW = 1


def _split_waits_pass(nc):
    n = 0
    for fn in nc.m.functions:
        for blk in fn.blocks:
            out = []
            changed = False
            for inst in blk.instructions:
                si = inst.sync_info
                waits = list(si.on_wait) if si is not None else []
                if len(waits) > _MAXW:
                    for i in range(0, len(waits) - _MAXW, _MAXW):
                        nop = mybir.InstNoOp(name=f"{inst.name}-wsplit{i}")
                        nop.engine = inst.engine
                        nop.sync_info = mybir.SyncInfo(
                            on_update=[], on_wait=waits[i : i + _MAXW]
                        )
                        out.append(nop)
                        n += 1
                    si.on_wait = waits[len(waits) - _MAXW :]
                    changed = True
                out.append(inst)
            if changed:
                blk.instructions = out
    return n


# concourse pins --enable-ldw-opt=false; enabling lets walrus elide/overlap
# redundant weight loads (the fp8 indicator weights repeat 6x per sample).
def _patch_ldw_opt():
    from concourse import bass_utils as _bu

    if getattr(_bu, "_ldw_opt_patched", False):
        return
    _orig = _bu.run_command

    def _run_command_ldwopt(cmd, *a, **kw):
        if isinstance(cmd, list):
            cmd = [
                "--enable-ldw-opt=true" if c == "--enable-ldw-opt=false" else c
                for c in cmd
            ]
        return _orig(cmd, *a, **kw)

    _bu.run_command = _run_command_ldwopt
    _bu._ldw_opt_patched = True


# NOTE: ldw-opt crashes walrus codegen on DoubleRow InstLdweights; leave off.
if os.environ.get("BASS_LDW_OPT", "0") == "1":
    _patch_ldw_opt()

F8 = mybir.dt.float8e4
U8 = mybir.dt.uint8
U16 = mybir.dt.uint16
U32 = mybir.dt.uint32
F32 = mybir.dt.float32
BF16 = mybir.dt.bfloat16

B, C, H, W = 64, 16, 32, 512
NCORES = 8
S = B // NCORES          # samples per core
R = C * H                # 512 rows in (c,h), c-major: r = c*32 + h
NB = R // 128            # 4 partition chunks
JB = W // 128            # 4 w-blocks
NW = 136                 # gram window = 128 + 2*4
MARGIN = 0.15
SHIFT = 4
WP = W + 8               # circularly padded row
# blob bytes/partition: a | pn (padded rows) | a-mask | pn-masks (padded)
XB = 2048 + 2 * NB * WP + 512 + 2 * WP
NLOAD = 4                # input loads in flight (SBUF tiles / DMA pressure)

_nc_cache = None


def build_nc(for_hw=True):
    DR = mybir.MatmulPerfMode.DoubleRow
    nc = bass.Bass()
    xin = nc.declare_dram_parameter("xin", [S, 128, XB], U8, isOutput=False)
    # indicator lhsT for the c-reduction: [k, i, m] = (m == k % 32), both i
    ind8 = nc.declare_dram_parameter("ind8", [128, 2, H], F8, isOutput=False)
    junk = nc.declare_dram_parameter("junk", [128, 2, W], F8, isOutput=False)
    # t3 gram bands; host reads the 9 diagonals col = m + 4 - off
    raw = nc.declare_dram_parameter("raw", [S, 128, 2, NW], BF16, isOutput=True)
    # P-maps: [h, (P1|P2p|P2n), w] = sum_c f^2, unmasked
    pmaps = nc.declare_dram_parameter("pmaps", [S, H, 3, W], BF16, isOutput=True)

    mult = mybir.AluOpType.mult
    band = mybir.AluOpType.bitwise_and
    Sq = mybir.ActivationFunctionType.Square
    Cp = mybir.ActivationFunctionType.Copy

    with tile.TileContext(nc) as tc, ExitStack() as ctx:
        const = ctx.enter_context(tc.tile_pool(name="const", bufs=1))
        io = ctx.enter_context(tc.tile_pool(name="io", bufs=1))
        um = ctx.enter_context(tc.tile_pool(name="um", bufs=3))
        sq = ctx.enter_context(tc.tile_pool(name="sq", bufs=3))
        outsb = ctx.enter_context(tc.tile_pool(name="outsb", bufs=3))
        indps = ctx.enter_context(tc.tile_pool(name="indps", bufs=2, space="PSUM"))
        gram = ctx.enter_context(tc.tile_pool(name="gram", bufs=2, space="PSUM"))

        ind_sb = const.tile([128, 2, H], F8)
        nc.sync.dma_start(out=ind_sb, in_=ind8[:])
        junk_sb = const.tile([128, 2, W], F8)
        nc.sync.dma_start(out=junk_sb, in_=junk[:])

        # Stagger sample loads: keep NLOAD in flight so DMA writes don't
        # saturate SBUF while engines read, but loads never wait on outputs.
        xts = [
            io.tile([128, XB], U8, tag=f"xt{s % NLOAD}", name=f"xt_s{s}")
            for s in range(S)
        ]
        for s in range(NLOAD - 1):
            nc.sync.dma_start(out=xts[s], in_=xin[s])

        # PE prewarm: dense fp8 matmuls (no cross deps beyond 2-buf WAW) so
        # the clock ramps during the pipeline-fill phase.
        for _ in range(24):
            wt = indps.tile([H, 3, W], F32, tag="pm")
            nc.tensor.matmul(
                wt[:, 0, :], ind_sb, junk_sb, start=True, stop=True, perf_mode=DR
            )

        PNO = 2048           # pn region offset
        AMO = 2048 + 2 * NB * WP   # a-mask offset
        for s in range(S):
            nxt = s + NLOAD - 1
            if nxt < S:
                nc.sync.dma_start(out=xts[nxt], in_=xin[nxt])
            xt = xts[s]
            a_f8 = xt[:, 0:2048].bitcast(F8).rearrange("p (j w) -> p j w", j=NB)
            pn_f8 = xt[:, PNO:AMO].bitcast(F8).rearrange(
                "p (t j w) -> p t j w", t=2, j=NB
            )
            a_u32 = xt[:, 0:2048].bitcast(U32).rearrange("p (j w) -> p j w", j=NB)
            pn_u32 = xt[:, PNO:AMO].bitcast(U32).rearrange(
                "p (t j w) -> p j t w", t=2, j=NB
            )
            amsk = xt[:, AMO : AMO + 512].bitcast(U32)
            pnmsk = xt[:, AMO + 512 : XB].bitcast(U32).rearrange(
                "p (r w) -> p r w", r=2
            )

            # ---- masking: bitwise AND on uint32 views (4 fp8/cycle);
            # pn rows come host-padded, so the gram window wrap is free ----
            ubuf = um.tile([128, NB, W], F8, tag="ubuf")
            vw = um.tile([128, NB, 2, WP], F8, tag="vw")
            nc.vector.tensor_tensor(
                out=ubuf.bitcast(U32),
                in0=a_u32,
                in1=amsk.unsqueeze(1).broadcast_to((128, NB, W // 4)),
                op=band,
            )
            nc.vector.tensor_tensor(
                out=vw.bitcast(U32),
                in0=pn_u32,
                in1=pnmsk.unsqueeze(1).broadcast_to((128, NB, 2, WP // 4)),
                op=band,
            )

            # ---- t3 gram: DoubleRow; per-pair matmuls share lhs weights ----
            num_ps = gram.tile([128, 2, NW], F32, tag="num")
            for j1 in range(JB):
                mb = slice(j1 * 128, (j1 + 1) * 128)
                wn = slice(j1 * 128, j1 * 128 + NW)
                for q in range(2):
                    for g in range(2):
                        nc.tensor.matmul(
                            num_ps[:, g, :],
                            ubuf[:, 2 * q : 2 * q + 2, mb],
                            vw[:, 2 * q : 2 * q + 2, g, wn],
                            start=(j1 == 0 and q == 0 and g == 0),
                            stop=(j1 == JB - 1 and q == 1 and g == 1),
                            perf_mode=DR,
                        )

            # ---- masked squares (Q = mask*P folds masks into the maps; the
            # host then needs only one mask factor per term). Reading
            # ubuf/vw keeps engines off the DMA-hot xt tile. ----
            u2 = sq.tile([128, NB, W], F8, tag="u2")
            vw2 = sq.tile([128, NB, 2, WP], F8, tag="vw2")
            nc.scalar.activation(out=u2, in_=ubuf, func=Sq)
            nc.scalar.activation(out=vw2[:, :, 0, :], in_=vw[:, :, 0, :], func=Sq)
            nc.gpsimd.tensor_tensor(
                out=vw2[:, :, 1, :], in0=vw[:, :, 1, :], in1=vw[:, :, 1, :], op=mult
            )

            # ---- c-reduction: DoubleRow indicator matmuls -> Q-maps ----
            pm_ps = indps.tile([H, 3, W], F32, tag="pm")
            for q in range(2):
                nc.tensor.matmul(
                    pm_ps[:, 0, :],
                    ind_sb,
                    u2[:, 2 * q : 2 * q + 2, :],
                    start=(q == 0),
                    stop=(q == 1),
                    perf_mode=DR,
                )
            for g in range(2):
                for q in range(2):
                    nc.tensor.matmul(
                        pm_ps[:, 1 + g, :],
                        ind_sb,
                        vw2[:, 2 * q : 2 * q + 2, g, 4 : W + 4],
                        start=(q == 0),
                        stop=(q == 1),
                        perf_mode=DR,
                    )

            # ---- PSUM -> SBUF bf16 (halves out-DMA bytes), then DRAM ----
            psb = outsb.tile([128, 2, NW], BF16, tag="rawsb")
            pmb = outsb.tile([H, 3, W], BF16, tag="pmsb")
            nc.scalar.activation(out=psb, in_=num_ps, func=Cp)
            nc.vector.tensor_copy(out=pmb, in_=pm_ps)
            nc.sync.dma_start(out=raw[s], in_=psb)
            nc.sync.dma_start(out=pmaps[s], in_=pmb)
    if for_hw:
        _split_waits_pass(nc)
    return nc


def _host_prep(a, p, n, ma, mp, mn):
    f8 = ml_dtypes.float8_e4m3

    def pack(x, pad):
        # [B, C, H, W] f32 -> fp8 bytes; partition p holds rows r = j*128+p,
        # j-major along the free dim, each row optionally circularly padded
        xr = np.asarray(x).reshape(B, R, W).astype(f8)
        if pad:
            xr = np.concatenate([xr[:, :, W - 4 :], xr, xr[:, :, :4]], axis=2)
        wp = xr.shape[-1]
        return np.ascontiguousarray(
            xr.reshape(B, NB, 128, wp).transpose(0, 2, 1, 3)
        ).reshape(B, 128, NB * wp).view(np.uint8)

    def mbytes(m, pad):
        mm = (np.asarray(m).reshape(B, H, W) != 0).astype(np.uint8) * np.uint8(0xFF)
        if pad:
            mm = np.concatenate([mm[:, :, W - 4 :], mm, mm[:, :, :4]], axis=2)
        return np.tile(mm, (1, NB, 1))  # [B, 128, wp], row p -> mask[p % 32]

    blob = np.concatenate(
        [
            pack(a, False),
            pack(p, True),
            pack(n, True),
            mbytes(ma, False),
            mbytes(mp, True),
            mbytes(mn, True),
        ],
        axis=2,
    )  # [B, 128, XB] u8
    ind8 = np.zeros((128, 2, H), f8)
    ind8[np.arange(128), :, np.arange(128) % H] = f8(1.0)
    junk = np.zeros((128, 2, W), f8)
    in_maps = []
    for c in range(NCORES):
        sl = slice(c * S, (c + 1) * S)
        in_maps.append({"xin": blob[sl], "ind8": ind8, "junk": junk})
    return in_maps


def _host_finish(raw_all, pm_all, ma, mp, mn):
    # raw_all [B, 128, 2, NW]: t3 band blocks (diag col = m + 4 - off)
    # pm_all  [B, H, 3, W]: masked maps Q1 = sum_c U^2, Q2g = sum_c Vg^2
    nb = raw_all.shape[0]
    raw64 = raw_all.astype(np.float64)
    pm64 = pm_all.astype(np.float64)
    m1 = np.asarray(ma).reshape(nb, H, W).astype(np.float64)
    m1b = m1.astype(bool)
    m2s = [np.asarray(mp).reshape(nb, H, W).astype(bool),
           np.asarray(mn).reshape(nb, H, W).astype(bool)]
    Q1 = pm64[:, :, 0, :]
    Q2s = [pm64[:, :, 1, :], pm64[:, :, 2, :]]
    idx = np.arange(128)
    dists = np.empty((2 * SHIFT + 1, nb, 2), np.float64)
    for i, off in enumerate(range(-SHIFT, SHIFT + 1)):
        t3 = raw64[:, idx, :, idx + 4 - off].sum(axis=0)  # [nb, 2]
        for g in range(2):
            m2r = np.roll(m2s[g], off, axis=-1)
            cnt = (m1b & m2r).sum(axis=(1, 2)).astype(np.float64)
            t1 = np.einsum("bhw,bhw->b", Q1, m2r.astype(np.float64))
            t2 = np.einsum("bhw,bhw->b", np.roll(Q2s[g], off, axis=-1), m1)
            num = t1 + t2 - 2.0 * t3[:, g]
            dists[i, :, g] = num / (C * cnt + 0.001)
    d = dists.min(axis=0)  # [nb, 2]
    loss = np.maximum(d[:, 0] - d[:, 1] + MARGIN, 0.0)
    return np.array(loss.mean(), dtype=np.float32)


def kernel(a, p, n, ma, mp, mn):
    global _nc_cache
    from concourse import bass_utils

    if _nc_cache is None:
        _nc_cache = build_nc()
    nc = _nc_cache
    in_maps = _host_prep(a, p, n, ma, mp, mn)
    res = bass_utils.run_bass_kernel_spmd(nc, in_maps, core_ids=list(range(NCORES)))
    raw_all = np.concatenate([res.results[i]["raw"] for i in range(NCORES)], axis=0)
    pm_all = np.concatenate([res.results[i]["pmaps"] for i in range(NCORES)], axis=0)
    return _host_finish(raw_all, pm_all, ma, mp, mn)


# revision 24
# speedup vs baseline: 1.0772x; 1.0772x over previous
"""Bass/Trainium2 kernel for ExtendedTripletLoss (data-parallel over batch).

FP8 redesign. Math per sample and shift off in [-4,4], pair g in {ap, an}:
  num(off) = t1 + t2 - 2*t3
    t3 = sum_{r,w} U[r,w] V[r,w-off],  U = m1*f1, V = m2*f2   (fp8)
    t1 = sum_{h,w} m1[h,w] m2[h,w-off] P1[h,w],   P1 = sum_c f1^2
    t2 = sum_{h,w} m1[h,w] m2[h,w-off] P2[h,w-off], P2 = sum_c f2^2
  den(off) = C * |m1 & m2r| + 1e-3

On-chip (per sample): one blob DMA carries fp8 a/p/n (chunk-major
[128, 4, 512]) plus 0xFF/0x00 mask bytes. Masking is a DVE bitwise-AND on
uint16 views (2x mode -> 4 fp8/cycle). Unmasked squares run split across
ScalarE/DVE/GpSimd. All matmuls are DoubleRow fp8 (K=256/step): an
indicator matmul reduces squares over c into P-maps [32, 3, 512], and the
U.V Gram accumulates band blocks into PSUM [128, 2, 136]. Both PSUM tiles
DMA straight to DRAM. Host extracts the 9 band diagonals (t3), applies
masks to the P-maps (t1/t2 - same O(B*H*W*9) class as the den counts),
and finishes min/relu/mean in f64.
"""

import os
import sys
from contextlib import ExitStack

import numpy as np

for _p in ("/opt/trn_rl_repo", "/root/.axon_site/_ro/trn_rl_repo"):
    if os.path.isdir(_p) and _p not in sys.path:
        sys.path.insert(0, _p)
        break

import ml_dtypes

import concourse.bass as bass
import concourse.mybir as mybir
import concourse.tile as tile

# This environment's walrus_driver allows only ONE sync-wait per instruction,
# while Tile freely aggregates several. Post-pass: move excess waits onto
# freshly inserted same-engine NOPs directly before the instruction.
_MAXW = 1


def _split_waits_pass(nc):
    n = 0
    for fn in nc.m.functions:
        for blk in fn.blocks:
            out = []
            changed = False
            for inst in blk.instructions:
                si = inst.sync_info
                waits = list(si.on_wait) if si is not None else []
                if len(waits) > _MAXW:
                    for i in range(0, len(waits) - _MAXW, _MAXW):
                        nop = mybir.InstNoOp(name=f"{inst.name}-wsplit{i}")
                        nop.engine = inst.engine
                        nop.sync_info = mybir.SyncInfo(
                            on_update=[], on_wait=waits[i : i + _MAXW]
                        )
                        out.append(nop)
                        n += 1
                    si.on_wait = waits[len(waits) - _MAXW :]
                    changed = True
                out.append(inst)
            if changed:
                blk.instructions = out
    return n


# concourse pins --enable-ldw-opt=false; enabling lets walrus elide/overlap
# redundant weight loads (the fp8 indicator weights repeat 6x per sample).
def _patch_ldw_opt():
    from concourse import bass_utils as _bu

    if getattr(_bu, "_ldw_opt_patched", False):
        return
    _orig = _bu.run_command

    def _run_command_ldwopt(cmd, *a, **kw):
        if isinstance(cmd, list):
            cmd = [
                "--enable-ldw-opt=true" if c == "--enable-ldw-opt=false" else c
                for c in cmd
            ]
        return _orig(cmd, *a, **kw)

    _bu.run_command = _run_command_ldwopt
    _bu._ldw_opt_patched = True


# NOTE: ldw-opt crashes walrus codegen on DoubleRow InstLdweights; leave off.
if os.environ.get("BASS_LDW_OPT", "0") == "1":
    _patch_ldw_opt()

F8 = mybir.dt.float8e4
U8 = mybir.dt.uint8
U16 = mybir.dt.uint16
U32 = mybir.dt.uint32
F32 = mybir.dt.float32
BF16 = mybir.dt.bfloat16

B, C, H, W = 64, 16, 32, 512
NCORES = 8
S = B // NCORES          # samples per core
R = C * H                # 512 rows in (c,h), c-major: r = c*32 + h
NB = R // 128            # 4 partition chunks
JB = W // 128            # 4 w-blocks
NW = 136                 # gram window = 128 + 2*4
MARGIN = 0.15
SHIFT = 4
WP = W + 8               # circularly padded row
# blob bytes/partition: a | pn (padded rows) | a-mask | pn-masks (padded)
XB = 2048 + 2 * NB * WP + 512 + 2 * WP
NLOAD = 4                # input loads in flight (SBUF tiles / DMA pressure)

_nc_cache = None


def build_nc(for_hw=True):
    DR = mybir.MatmulPerfMode.DoubleRow
    nc = bass.Bass()
    xin = nc.declare_dram_parameter("xin", [S, 128, XB], U8, isOutput=False)
    # indicator lhsT for the c-reduction: [k, i, m] = (m == k % 32), both i
    ind8 = nc.declare_dram_parameter("ind8", [128, 2, H], F8, isOutput=False)
    junk = nc.declare_dram_parameter("junk", [128, 2, W], F8, isOutput=False)
    # t3 gram bands; host reads the 9 diagonals col = m + 4 - off
    raw = nc.declare_dram_parameter("raw", [S, 128, 2, NW], BF16, isOutput=True)
    # P-maps: [h, (P1|P2p|P2n), w] = sum_c f^2, unmasked
    pmaps = nc.declare_dram_parameter("pmaps", [S, H, 3, W], BF16, isOutput=True)

    mult = mybir.AluOpType.mult
    band = mybir.AluOpType.bitwise_and
    Sq = mybir.ActivationFunctionType.Square
    Cp = mybir.ActivationFunctionType.Copy

    with tile.TileContext(nc) as tc, ExitStack() as ctx:
        const = ctx.enter_context(tc.tile_pool(name="const", bufs=1))
        io = ctx.enter_context(tc.tile_pool(name="io", bufs=1))
        um = ctx.enter_context(tc.tile_pool(name="um", bufs=3))
        sq = ctx.enter_context(tc.tile_pool(name="sq", bufs=3))
        outsb = ctx.enter_context(tc.tile_pool(name="outsb", bufs=3))
        indps = ctx.enter_context(tc.tile_pool(name="indps", bufs=2, space="PSUM"))
        gram = ctx.enter_context(tc.tile_pool(name="gram", bufs=2, space="PSUM"))

        ind_sb = const.tile([128, 2, H], F8)
        nc.sync.dma_start(out=ind_sb, in_=ind8[:])
        junk_sb = const.tile([128, 2, W], F8)
        nc.sync.dma_start(out=junk_sb, in_=junk[:])

        # Stagger sample loads: keep NLOAD in flight so DMA writes don't
        # saturate SBUF while engines read, but loads never wait on outputs.
        xts = [
            io.tile([128, XB], U8, tag=f"xt{s % NLOAD}", name=f"xt_s{s}")
            for s in range(S)
        ]
        for s in range(NLOAD - 1):
            nc.sync.dma_start(out=xts[s], in_=xin[s])

        # PE prewarm: dense fp8 matmuls (no cross deps beyond 2-buf WAW) so
        # the clock ramps during the pipeline-fill phase.
        for _ in range(24):
            wt = indps.tile([H, 3, W], F32, tag="pm")
            nc.tensor.matmul(
                wt[:, 0, :], ind_sb, junk_sb, start=True, stop=True, perf_mode=DR
            )

        PNO = 2048           # pn region offset
        AMO = 2048 + 2 * NB * WP   # a-mask offset
        for s in range(S):
            nxt = s + NLOAD - 1
            if nxt < S:
                nc.sync.dma_start(out=xts[nxt], in_=xin[nxt])
            xt = xts[s]
            a_f8 = xt[:, 0:2048].bitcast(F8).rearrange("p (j w) -> p j w", j=NB)
            pn_f8 = xt[:, PNO:AMO].bitcast(F8).rearrange(
                "p (t j w) -> p t j w", t=2, j=NB
            )
            a_u32 = xt[:, 0:2048].bitcast(U32).rearrange("p (j w) -> p j w", j=NB)
            pn_u32 = xt[:, PNO:AMO].bitcast(U32).rearrange(
                "p (t j w) -> p j t w", t=2, j=NB
            )
            amsk = xt[:, AMO : AMO + 512].bitcast(U32)
            pnmsk = xt[:, AMO + 512 : XB].bitcast(U32).rearrange(
                "p (r w) -> p r w", r=2
            )

            # ---- masking: bitwise AND on uint32 views (4 fp8/cycle);
            # pn rows come host-padded, so the gram window wrap is free ----
            ubuf = um.tile([128, NB, W], F8, tag="ubuf")
            vw = um.tile([128, NB, 2, WP], F8, tag="vw")
            nc.vector.tensor_tensor(
                out=ubuf.bitcast(U32),
                in0=a_u32,
                in1=amsk.unsqueeze(1).broadcast_to((128, NB, W // 4)),
                op=band,
            )
            nc.vector.tensor_tensor(
                out=vw.bitcast(U32),
                in0=pn_u32,
                in1=pnmsk.unsqueeze(1).broadcast_to((128, NB, 2, WP // 4)),
                op=band,
            )

            # ---- t3 gram: DoubleRow; per-pair matmuls share lhs weights ----
            num_ps = gram.tile([128, 2, NW], F32, tag="num")
            for j1 in range(JB):
                mb = slice(j1 * 128, (j1 + 1) * 128)
                wn = slice(j1 * 128, j1 * 128 + NW)
                for q in range(2):
                    for g in range(2):
                        nc.tensor.matmul(
                            num_ps[:, g, :],
                            ubuf[:, 2 * q : 2 * q + 2, mb],
                            vw[:, 2 * q : 2 * q + 2, g, wn],
                            start=(j1 == 0 and q == 0 and g == 0),
                            stop=(j1 == JB - 1 and q == 1 and g == 1),
                            perf_mode=DR,
                        )

            # ---- unmasked squares from contiguous blob regions:
            # ACT a^2 + p^2, GpSimd n^2 (padded rows, one op each) ----
            u2 = sq.tile([128, NB, W], F8, tag="u2")
            p2 = sq.tile([128, NB, WP], F8, tag="p2")
            n2 = sq.tile([128, NB, WP], F8, tag="n2")
            nc.gpsimd.tensor_tensor(
                out=n2, in0=pn_f8[:, 1], in1=pn_f8[:, 1], op=mult
            )
            nc.scalar.activation(out=u2, in_=a_f8, func=Sq)
            nc.scalar.activation(out=p2, in_=pn_f8[:, 0], func=Sq)

            # ---- c-reduction: DoubleRow indicator matmuls -> P-maps ----
            pm_ps = indps.tile([H, 3, W], F32, tag="pm")
            for t, (sqt, off0) in enumerate(((u2, 0), (p2, 4), (n2, 4))):
                for q in range(2):
                    nc.tensor.matmul(
                        pm_ps[:, t, :],
                        ind_sb,
                        sqt[:, 2 * q : 2 * q + 2, off0 : off0 + W],
                        start=(q == 0),
                        stop=(q == 1),
                        perf_mode=DR,
                    )

            # ---- PSUM -> SBUF bf16 (halves out-DMA bytes), then DRAM ----
            psb = outsb.tile([128, 2, NW], BF16, tag="rawsb")
            pmb = outsb.tile([H, 3, W], BF16, tag="pmsb")
            nc.scalar.activation(out=psb, in_=num_ps, func=Cp)
            nc.scalar.activation(out=pmb[:, 0, :], in_=pm_ps[:, 0, :], func=Cp)
            nc.vector.tensor_copy(out=pmb[:, 1:3, :], in_=pm_ps[:, 1:3, :])
            nc.sync.dma_start(out=raw[s], in_=psb)
            nc.sync.dma_start(out=pmaps[s], in_=pmb)
    if for_hw:
        _split_waits_pass(nc)
    return nc


def _host_prep(a, p, n, ma, mp, mn):
    f8 = ml_dtypes.float8_e4m3

    def pack(x, pad):
        # [B, C, H, W] f32 -> fp8 bytes; partition p holds rows r = j*128+p,
        # j-major along the free dim, each row optionally circularly padded
        xr = np.asarray(x).reshape(B, R, W).astype(f8)
        if pad:
            xr = np.concatenate([xr[:, :, W - 4 :], xr, xr[:, :, :4]], axis=2)
        wp = xr.shape[-1]
        return np.ascontiguousarray(
            xr.reshape(B, NB, 128, wp).transpose(0, 2, 1, 3)
        ).reshape(B, 128, NB * wp).view(np.uint8)

    def mbytes(m, pad):
        mm = (np.asarray(m).reshape(B, H, W) != 0).astype(np.uint8) * np.uint8(0xFF)
        if pad:
            mm = np.concatenate([mm[:, :, W - 4 :], mm, mm[:, :, :4]], axis=2)
        return np.tile(mm, (1, NB, 1))  # [B, 128, wp], row p -> mask[p % 32]

    blob = np.concatenate(
        [
            pack(a, False),
            pack(p, True),
            pack(n, True),
            mbytes(ma, False),
            mbytes(mp, True),
            mbytes(mn, True),
        ],
        axis=2,
    )  # [B, 128, XB] u8
    ind8 = np.zeros((128, 2, H), f8)
    ind8[np.arange(128), :, np.arange(128) % H] = f8(1.0)
    junk = np.zeros((128, 2, W), f8)
    in_maps = []
    for c in range(NCORES):
        sl = slice(c * S, (c + 1) * S)
        in_maps.append({"xin": blob[sl], "ind8": ind8, "junk": junk})
    return in_maps


def _host_finish(raw_all, pm_all, ma, mp, mn):
    # raw_all [B, 128, 2, NW]: t3 band blocks (diag col = m + 4 - off)
    # pm_all  [B, H, 3, W]: unmasked maps P1, P2p, P2n = sum_c f^2
    nb = raw_all.shape[0]
    raw64 = raw_all.astype(np.float64)
    pm64 = pm_all.astype(np.float64)
    m1 = np.asarray(ma).reshape(nb, H, W).astype(bool)
    m2s = [np.asarray(mp).reshape(nb, H, W).astype(bool),
           np.asarray(mn).reshape(nb, H, W).astype(bool)]
    P1 = pm64[:, :, 0, :]
    P2s = [pm64[:, :, 1, :], pm64[:, :, 2, :]]
    idx = np.arange(128)
    dists = np.empty((2 * SHIFT + 1, nb, 2), np.float64)
    for i, off in enumerate(range(-SHIFT, SHIFT + 1)):
        t3 = raw64[:, idx, :, idx + 4 - off].sum(axis=0)  # [nb, 2]
        for g in range(2):
            m2r = np.roll(m2s[g], off, axis=-1)
            both = (m1 & m2r).astype(np.float64)
            cnt = both.sum(axis=(1, 2))
            t1 = np.einsum("bhw,bhw->b", P1, both)
            t2 = np.einsum("bhw,bhw->b", np.roll(P2s[g], off, axis=-1), both)
            num = t1 + t2 - 2.0 * t3[:, g]
            dists[i, :, g] = num / (C * cnt + 0.001)
    d = dists.min(axis=0)  # [nb, 2]
    loss = np.maximum(d[:, 0] - d[:, 1] + MARGIN, 0.0)
    return np.array(loss.mean(), dtype=np.float32)


def kernel(a, p, n, ma, mp, mn):
    global _nc_cache
    from concourse import bass_utils

    if _nc_cache is None:
        _nc_cache = build_nc()
    nc = _nc_cache
    in_maps = _host_prep(a, p, n, ma, mp, mn)
    res = bass_utils.run_bass_kernel_spmd(nc, in_maps, core_ids=list(range(NCORES)))
    raw_all = np.concatenate([res.results[i]["raw"] for i in range(NCORES)], axis=0)
    pm_all = np.concatenate([res.results[i]["pmaps"] for i in range(NCORES)], axis=0)
    return _host_finish(raw_all, pm_all, ma, mp, mn)


# revision 25
# speedup vs baseline: 1.1312x; 1.0502x over previous
"""Bass/Trainium2 kernel for ExtendedTripletLoss (data-parallel over batch).

FP8 redesign. Math per sample and shift off in [-4,4], pair g in {ap, an}:
  num(off) = t1 + t2 - 2*t3
    t3 = sum_{r,w} U[r,w] V[r,w-off],  U = m1*f1, V = m2*f2   (fp8)
    t1 = sum_{h,w} m1[h,w] m2[h,w-off] P1[h,w],   P1 = sum_c f1^2
    t2 = sum_{h,w} m1[h,w] m2[h,w-off] P2[h,w-off], P2 = sum_c f2^2
  den(off) = C * |m1 & m2r| + 1e-3

On-chip (per sample): one blob DMA carries fp8 a/p/n (chunk-major
[128, 4, 512]) plus 0xFF/0x00 mask bytes. Masking is a DVE bitwise-AND on
uint16 views (2x mode -> 4 fp8/cycle). Unmasked squares run split across
ScalarE/DVE/GpSimd. All matmuls are DoubleRow fp8 (K=256/step): an
indicator matmul reduces squares over c into P-maps [32, 3, 512], and the
U.V Gram accumulates band blocks into PSUM [128, 2, 136]. Both PSUM tiles
DMA straight to DRAM. Host extracts the 9 band diagonals (t3), applies
masks to the P-maps (t1/t2 - same O(B*H*W*9) class as the den counts),
and finishes min/relu/mean in f64.
"""

import os
import sys
from contextlib import ExitStack

import numpy as np

for _p in ("/opt/trn_rl_repo", "/root/.axon_site/_ro/trn_rl_repo"):
    if os.path.isdir(_p) and _p not in sys.path:
        sys.path.insert(0, _p)
        break

import ml_dtypes

import concourse.bass as bass
import concourse.mybir as mybir
import concourse.tile as tile

# This environment's walrus_driver allows only ONE sync-wait per instruction,
# while Tile freely aggregates several. Post-pass: move excess waits onto
# freshly inserted same-engine NOPs directly before the instruction.
_MAXW = 1


def _split_waits_pass(nc):
    n = 0
    for fn in nc.m.functions:
        for blk in fn.blocks:
            out = []
            changed = False
            for inst in blk.instructions:
                si = inst.sync_info
                waits = list(si.on_wait) if si is not None else []
                if len(waits) > _MAXW:
                    for i in range(0, len(waits) - _MAXW, _MAXW):
                        nop = mybir.InstNoOp(name=f"{inst.name}-wsplit{i}")
                        nop.engine = inst.engine
                        nop.sync_info = mybir.SyncInfo(
                            on_update=[], on_wait=waits[i : i + _MAXW]
                        )
                        out.append(nop)
                        n += 1
                    si.on_wait = waits[len(waits) - _MAXW :]
                    changed = True
                out.append(inst)
            if changed:
                blk.instructions = out
    return n


# concourse pins --enable-ldw-opt=false; enabling lets walrus elide/overlap
# redundant weight loads (the fp8 indicator weights repeat 6x per sample).
def _patch_ldw_opt():
    from concourse import bass_utils as _bu

    if getattr(_bu, "_ldw_opt_patched", False):
        return
    _orig = _bu.run_command

    def _run_command_ldwopt(cmd, *a, **kw):
        if isinstance(cmd, list):
            cmd = [
                "--enable-ldw-opt=true" if c == "--enable-ldw-opt=false" else c
                for c in cmd
            ]
        return _orig(cmd, *a, **kw)

    _bu.run_command = _run_command_ldwopt
    _bu._ldw_opt_patched = True


# NOTE: ldw-opt crashes walrus codegen on DoubleRow InstLdweights; leave off.
if os.environ.get("BASS_LDW_OPT", "0") == "1":
    _patch_ldw_opt()

F8 = mybir.dt.float8e4
U8 = mybir.dt.uint8
U16 = mybir.dt.uint16
U32 = mybir.dt.uint32
F32 = mybir.dt.float32
BF16 = mybir.dt.bfloat16

B, C, H, W = 64, 16, 32, 512
NCORES = 8
S = B // NCORES          # samples per core
R = C * H                # 512 rows in (c,h), c-major: r = c*32 + h
NB = R // 128            # 4 partition chunks
JB = W // 128            # 4 w-blocks
NW = 136                 # gram window = 128 + 2*4
MARGIN = 0.15
SHIFT = 4
WP = W + 8               # circularly padded row
# blob bytes/partition: a | pn (padded rows) | a-mask | pn-masks (padded)
XB = 2048 + 2 * NB * WP + 512 + 2 * WP
NLOAD = 4                # input loads in flight (SBUF tiles / DMA pressure)

_nc_cache = None


def build_nc(for_hw=True):
    DR = mybir.MatmulPerfMode.DoubleRow
    nc = bass.Bass()
    xin = nc.declare_dram_parameter("xin", [S, 128, XB], U8, isOutput=False)
    # indicator lhsT for the c-reduction: [k, i, m] = (m == k % 32), both i
    ind8 = nc.declare_dram_parameter("ind8", [128, 2, H], F8, isOutput=False)
    junk = nc.declare_dram_parameter("junk", [128, 2, W], F8, isOutput=False)
    # t3 gram bands; host reads the 9 diagonals col = m + 4 - off
    raw = nc.declare_dram_parameter("raw", [S, 128, 2, NW], BF16, isOutput=True)
    # P-maps: [h, (P1|P2p|P2n), w] = sum_c f^2, unmasked
    pmaps = nc.declare_dram_parameter("pmaps", [S, H, 3, W], BF16, isOutput=True)

    mult = mybir.AluOpType.mult
    band = mybir.AluOpType.bitwise_and
    Sq = mybir.ActivationFunctionType.Square
    Cp = mybir.ActivationFunctionType.Copy

    with tile.TileContext(nc) as tc, ExitStack() as ctx:
        const = ctx.enter_context(tc.tile_pool(name="const", bufs=1))
        io = ctx.enter_context(tc.tile_pool(name="io", bufs=1))
        um = ctx.enter_context(tc.tile_pool(name="um", bufs=4))
        sq = ctx.enter_context(tc.tile_pool(name="sq", bufs=4))
        outsb = ctx.enter_context(tc.tile_pool(name="outsb", bufs=3))
        indps = ctx.enter_context(tc.tile_pool(name="indps", bufs=2, space="PSUM"))
        gram = ctx.enter_context(tc.tile_pool(name="gram", bufs=2, space="PSUM"))

        ind_sb = const.tile([128, 2, H], F8)
        nc.sync.dma_start(out=ind_sb, in_=ind8[:])
        junk_sb = const.tile([128, 2, W], F8)
        nc.sync.dma_start(out=junk_sb, in_=junk[:])

        # Stagger sample loads: keep NLOAD in flight so DMA writes don't
        # saturate SBUF while engines read, but loads never wait on outputs.
        xts = [
            io.tile([128, XB], U8, tag=f"xt{s % NLOAD}", name=f"xt_s{s}")
            for s in range(S)
        ]
        for s in range(NLOAD - 1):
            nc.sync.dma_start(out=xts[s], in_=xin[s])

        # PE prewarm: dense fp8 matmuls (no cross deps beyond 2-buf WAW) so
        # the clock ramps during the pipeline-fill phase.
        for _ in range(24):
            wt = indps.tile([H, 3, W], F32, tag="pm")
            nc.tensor.matmul(
                wt[:, 0, :], ind_sb, junk_sb, start=True, stop=True, perf_mode=DR
            )

        PNO = 2048           # pn region offset
        AMO = 2048 + 2 * NB * WP   # a-mask offset
        for s in range(S):
            nxt = s + NLOAD - 1
            if nxt < S:
                nc.sync.dma_start(out=xts[nxt], in_=xin[nxt])
            xt = xts[s]
            a_f8 = xt[:, 0:2048].bitcast(F8).rearrange("p (j w) -> p j w", j=NB)
            pn_f8 = xt[:, PNO:AMO].bitcast(F8).rearrange(
                "p (t j w) -> p t j w", t=2, j=NB
            )
            a_u32 = xt[:, 0:2048].bitcast(U32).rearrange("p (j w) -> p j w", j=NB)
            pn_u32 = xt[:, PNO:AMO].bitcast(U32).rearrange(
                "p (t j w) -> p j t w", t=2, j=NB
            )
            amsk = xt[:, AMO : AMO + 512].bitcast(U32)
            pnmsk = xt[:, AMO + 512 : XB].bitcast(U32).rearrange(
                "p (r w) -> p r w", r=2
            )

            # ---- masking: bitwise AND on uint32 views (4 fp8/cycle);
            # pn rows come host-padded, so the gram window wrap is free ----
            ubuf = um.tile([128, NB, W], F8, tag="ubuf")
            vw = um.tile([128, NB, 2, WP], F8, tag="vw")
            nc.vector.tensor_tensor(
                out=ubuf.bitcast(U32),
                in0=a_u32,
                in1=amsk.unsqueeze(1).broadcast_to((128, NB, W // 4)),
                op=band,
            )
            nc.vector.tensor_tensor(
                out=vw.bitcast(U32),
                in0=pn_u32,
                in1=pnmsk.unsqueeze(1).broadcast_to((128, NB, 2, WP // 4)),
                op=band,
            )

            # ---- t3 gram: DoubleRow; per-pair matmuls share lhs weights ----
            num_ps = gram.tile([128, 2, NW], F32, tag="num")
            for j1 in range(JB):
                mb = slice(j1 * 128, (j1 + 1) * 128)
                wn = slice(j1 * 128, j1 * 128 + NW)
                for q in range(2):
                    for g in range(2):
                        nc.tensor.matmul(
                            num_ps[:, g, :],
                            ubuf[:, 2 * q : 2 * q + 2, mb],
                            vw[:, 2 * q : 2 * q + 2, g, wn],
                            start=(j1 == 0 and q == 0 and g == 0),
                            stop=(j1 == JB - 1 and q == 1 and g == 1),
                            perf_mode=DR,
                        )

            # ---- unmasked squares from contiguous blob regions:
            # ACT a^2 + p^2, GpSimd n^2 (padded rows, one op each) ----
            u2 = sq.tile([128, NB, W], F8, tag="u2")
            p2 = sq.tile([128, NB, WP], F8, tag="p2")
            n2 = sq.tile([128, NB, WP], F8, tag="n2")
            nc.gpsimd.tensor_tensor(
                out=n2, in0=pn_f8[:, 1], in1=pn_f8[:, 1], op=mult
            )
            nc.scalar.activation(out=u2, in_=a_f8, func=Sq)
            nc.scalar.activation(out=p2, in_=pn_f8[:, 0], func=Sq)

            # ---- c-reduction: DoubleRow indicator matmuls -> P-maps ----
            pm_ps = indps.tile([H, 3, W], F32, tag="pm")
            for t, (sqt, off0) in enumerate(((u2, 0), (p2, 4), (n2, 4))):
                for q in range(2):
                    nc.tensor.matmul(
                        pm_ps[:, t, :],
                        ind_sb,
                        sqt[:, 2 * q : 2 * q + 2, off0 : off0 + W],
                        start=(q == 0),
                        stop=(q == 1),
                        perf_mode=DR,
                    )

            # ---- PSUM -> SBUF bf16 (halves out-DMA bytes), then DRAM ----
            psb = outsb.tile([128, 2, NW], BF16, tag="rawsb")
            pmb = outsb.tile([H, 3, W], BF16, tag="pmsb")
            nc.scalar.activation(out=psb, in_=num_ps, func=Cp)
            nc.scalar.activation(out=pmb[:, 0, :], in_=pm_ps[:, 0, :], func=Cp)
            nc.vector.tensor_copy(out=pmb[:, 1:3, :], in_=pm_ps[:, 1:3, :])
            nc.sync.dma_start(out=raw[s], in_=psb)
            nc.sync.dma_start(out=pmaps[s], in_=pmb)
    if for_hw:
        _split_waits_pass(nc)
    return nc


def _host_prep(a, p, n, ma, mp, mn):
    f8 = ml_dtypes.float8_e4m3

    def pack(x, pad):
        # [B, C, H, W] f32 -> fp8 bytes; partition p holds rows r = j*128+p,
        # j-major along the free dim, each row optionally circularly padded
        xr = np.asarray(x).reshape(B, R, W).astype(f8)
        if pad:
            xr = np.concatenate([xr[:, :, W - 4 :], xr, xr[:, :, :4]], axis=2)
        wp = xr.shape[-1]
        return np.ascontiguousarray(
            xr.reshape(B, NB, 128, wp).transpose(0, 2, 1, 3)
        ).reshape(B, 128, NB * wp).view(np.uint8)

    def mbytes(m, pad):
        mm = (np.asarray(m).reshape(B, H, W) != 0).astype(np.uint8) * np.uint8(0xFF)
        if pad:
            mm = np.concatenate([mm[:, :, W - 4 :], mm, mm[:, :, :4]], axis=2)
        return np.tile(mm, (1, NB, 1))  # [B, 128, wp], row p -> mask[p % 32]

    blob = np.concatenate(
        [
            pack(a, False),
            pack(p, True),
            pack(n, True),
            mbytes(ma, False),
            mbytes(mp, True),
            mbytes(mn, True),
        ],
        axis=2,
    )  # [B, 128, XB] u8
    ind8 = np.zeros((128, 2, H), f8)
    ind8[np.arange(128), :, np.arange(128) % H] = f8(1.0)
    junk = np.zeros((128, 2, W), f8)
    in_maps = []
    for c in range(NCORES):
        sl = slice(c * S, (c + 1) * S)
        in_maps.append({"xin": blob[sl], "ind8": ind8, "junk": junk})
    return in_maps


def _host_finish(raw_all, pm_all, ma, mp, mn):
    # raw_all [B, 128, 2, NW]: t3 band blocks (diag col = m + 4 - off)
    # pm_all  [B, H, 3, W]: unmasked maps P1, P2p, P2n = sum_c f^2
    nb = raw_all.shape[0]
    raw64 = raw_all.astype(np.float64)
    pm64 = pm_all.astype(np.float64)
    m1 = np.asarray(ma).reshape(nb, H, W).astype(bool)
    m2s = [np.asarray(mp).reshape(nb, H, W).astype(bool),
           np.asarray(mn).reshape(nb, H, W).astype(bool)]
    P1 = pm64[:, :, 0, :]
    P2s = [pm64[:, :, 1, :], pm64[:, :, 2, :]]
    idx = np.arange(128)
    dists = np.empty((2 * SHIFT + 1, nb, 2), np.float64)
    for i, off in enumerate(range(-SHIFT, SHIFT + 1)):
        t3 = raw64[:, idx, :, idx + 4 - off].sum(axis=0)  # [nb, 2]
        for g in range(2):
            m2r = np.roll(m2s[g], off, axis=-1)
            both = (m1 & m2r).astype(np.float64)
            cnt = both.sum(axis=(1, 2))
            t1 = np.einsum("bhw,bhw->b", P1, both)
            t2 = np.einsum("bhw,bhw->b", np.roll(P2s[g], off, axis=-1), both)
            num = t1 + t2 - 2.0 * t3[:, g]
            dists[i, :, g] = num / (C * cnt + 0.001)
    d = dists.min(axis=0)  # [nb, 2]
    loss = np.maximum(d[:, 0] - d[:, 1] + MARGIN, 0.0)
    return np.array(loss.mean(), dtype=np.float32)


def kernel(a, p, n, ma, mp, mn):
    global _nc_cache
    from concourse import bass_utils

    if _nc_cache is None:
        _nc_cache = build_nc()
    nc = _nc_cache
    in_maps = _host_prep(a, p, n, ma, mp, mn)
    res = bass_utils.run_bass_kernel_spmd(nc, in_maps, core_ids=list(range(NCORES)))
    raw_all = np.concatenate([res.results[i]["raw"] for i in range(NCORES)], axis=0)
    pm_all = np.concatenate([res.results[i]["pmaps"] for i in range(NCORES)], axis=0)
    return _host_finish(raw_all, pm_all, ma, mp, mn)
